# revision 13
# baseline (speedup 1.0000x reference)
"""Trainium2 Bass kernel for nn_LRSVConv (low-rank spatially-varying conv).

Computes, for full inputs
    x            [8, 32, 256, 256]  f32
    conv_w       [192, 32, 3, 3]    f32   (192 = RANK(3) * C_OUT(64))
    kernel_weight[2, 256, 256]      f32
the reference:
    y   = conv2d(x, conv_w, stride 1, pad 1)      # [8, 192, 256, 256]
    y   = y.reshape(8, 3, 64, 256, 256)
    out = y[:,0] + kw[0]*y[:,1] + kw[1]*y[:,2]    # [8, 64, 256, 256]

Strategy: spatial (H) sharding across 8 cores - each core computes a band of
32 output rows for ALL batches, so the per-pixel blend weights (which are
batch-independent) are loaded/broadcast once per core and reused 8x.

Per core:
  - imcol tile [96, 32*258]: 3 kh-shifted replicas of the padded input rows
    (partition dim = (kh, c_in)), padded W=258 so kw shifts are free-dim
    offsets and no edge handling is needed.
  - conv: per supertile (4 output rows = 1024 px, split into 2 blocks of
    512 px), per rank r and kw: one K=96, M=64, N=512 fp32 matmul per block,
    the two blocks on opposite column halves of the PE array (concurrent via
    col tiling), accumulating in PSUM banks A/B/C (one per rank); psum rows
    = (block, c_out).
  - blend: t1 = B * sv1_bcast, t2 = C * sv2_bcast on DVE; t1 accumulated
    onto A via an identity matmul on the (otherwise busier) TensorE;
    out = A + t2 on DVE (fused PSUM evacuation).
  - sv broadcast tiles are prepared host-side ([128, 4096] per rank: rows
    (block, c) x band pixels) - tiny input, avoids on-device partition
    broadcast which no engine does well.
"""

import os

import numpy as np

B, C_IN, C_OUT, RANK, IMG = 8, 32, 64, 3, 256
N_CORES = 8
BAND = IMG // N_CORES          # 32 output rows per core
WP = IMG + 2                   # padded width 258
ROWS_IN = BAND + 2             # input rows needed per band (with halo)
SUPER = 8                      # supertiles per (batch, band): 4 rows each
SROWS = BAND // SUPER          # 4 image rows per supertile
NBLK = 512                     # pixels per matmul block (2 image rows)

_F32 = np.float32

# "pe": rank-1 partial added into PSUM A by an identity matmul on TensorE
# "dve": both adds on VectorE (simpler, more DVE load)
BLEND_MODE = os.environ.get("KERNEL_BLEND", "pe")
NB = int(os.environ.get("KERNEL_NB", str(B)))  # batches to process (debug knob)


def _build_bass():
    import concourse.mybir as mybir
    import concourse.tile as tile
    from concourse import bacc

    f32 = mybir.dt.float32
    nc = bacc.Bacc("TRN2", target_bir_lowering=False, debug=False)

    xs_t = nc.dram_tensor("xs", (B, C_IN, ROWS_IN * WP), f32, kind="ExternalInput")
    wt_t = nc.dram_tensor("wt", (96, 3, RANK * C_OUT), f32, kind="ExternalInput")
    svb_t = nc.dram_tensor("svb", (128, 2, SUPER * NBLK), f32, kind="ExternalInput")
    id_t = nc.dram_tensor("ident", (128, 128), f32, kind="ExternalInput")
    out_t = nc.dram_tensor("out", (B, C_OUT, BAND, IMG), f32, kind="ExternalOutput")

    xs = xs_t.ap()
    # out view: [b, q, c, supertile, 512] so the [128=(q c), 512] sbuf tile
    # can be stored with one DMA per (b, supertile)
    out_r = out_t.ap().rearrange(
        "b c (t q r) w -> b q c t (r w)", t=SUPER, q=2, r=SROWS // 2
    )

    with tile.TileContext(nc) as tc:
        with (
            tc.tile_pool(name="const", bufs=1) as cpool,
            tc.tile_pool(name="imcol", bufs=2) as ipool,
            tc.tile_pool(name="psum", bufs=2, space="PSUM") as ppool,
            tc.tile_pool(name="tmp", bufs=3) as tpool,
            tc.tile_pool(name="outp", bufs=4) as opool,
        ):
            wt_sb = cpool.tile([96, 3, RANK * C_OUT], f32)
            nc.sync.dma_start(wt_sb[:], wt_t.ap())
            svb_sb = cpool.tile([128, 2, SUPER * NBLK], f32)
            nc.sync.dma_start(svb_sb[:], svb_t.ap())
            id_sb = cpool.tile([128, 128], f32)
            nc.sync.dma_start(id_sb[:], id_t.ap())

            for b in range(NB):
                imcol = ipool.tile([96, BAND * WP], f32, tag="imcol")
                for kh in range(3):
                    # rows [kh, kh+BAND) of the 34-row padded slab, contiguous
                    nc.sync.dma_start(
                        imcol[32 * kh : 32 * kh + 32, :],
                        xs[b, :, kh * WP : kh * WP + BAND * WP],
                    )
                imv = imcol.rearrange("p (h w) -> p h w", w=WP)

                for t in range(SUPER):
                    ps = []
                    for r, tg in enumerate(("psA", "psB", "psC")):
                        p = ppool.tile([128, NBLK], f32, tag=tg)
                        ps.append(p)
                    for r in range(RANK):
                        for kw in range(3):
                            for q in range(2):
                                hl = SROWS * t + 2 * q
                                rhs = imv[:, hl : hl + 2, kw : kw + IMG]
                                # per 64-partition col-tile half: its own
                                # start/stop accumulation group
                                stop = (kw == 2) and not (
                                    r == 0 and BLEND_MODE == "pe"
                                )
                                # skip_group_check: the sim's group tracker
                                # conflates the two 64-partition halves of a
                                # bank; its per-partition pending-zero data
                                # model handles this split correctly.
                                nc.tensor.matmul(
                                    ps[r][64 * q : 64 * q + 64, :],
                                    wt_sb[:, kw, 64 * r : 64 * r + 64],
                                    rhs,
                                    start=(kw == 0),
                                    stop=stop,
                                    skip_group_check=True,
                                )

                    sv1 = svb_sb[:, 0, NBLK * t : NBLK * (t + 1)]
                    sv2 = svb_sb[:, 1, NBLK * t : NBLK * (t + 1)]
                    t1 = tpool.tile([128, NBLK], f32, tag="t1")
                    nc.vector.tensor_tensor(t1[:], ps[1], sv1, mybir.AluOpType.mult)
                    t2 = tpool.tile([128, NBLK], f32, tag="t2")
                    nc.vector.tensor_tensor(t2[:], ps[2], sv2, mybir.AluOpType.mult)

                    out_sb = opool.tile([128, NBLK], f32, tag="out_sb")
                    if BLEND_MODE == "pe":
                        # A += t1 on TensorE (identity matmul), then fused
                        # evacuate: out = A + t2 on DVE.
                        nc.tensor.matmul(
                            ps[0][:],
                            id_sb[:],
                            t1[:],
                            start=False,
                            stop=True,
                            skip_group_check=True,
                        )
                        nc.vector.tensor_tensor(
                            out_sb[:], ps[0], t2[:], mybir.AluOpType.add
                        )
                    else:
                        a1 = tpool.tile([128, NBLK], f32, tag="a1")
                        nc.vector.tensor_tensor(
                            a1[:], ps[0], t1[:], mybir.AluOpType.add
                        )
                        nc.vector.tensor_tensor(
                            out_sb[:], a1[:], t2[:], mybir.AluOpType.add
                        )

                    for q in range(2):
                        nc.sync.dma_start(
                            out_r[b, q, :, t, :], out_sb[64 * q : 64 * q + 64, :]
                        )
    nc.compile()
    return nc


_CACHE = {}


def _get_bass():
    if "nc" not in _CACHE:
        _CACHE["nc"] = _build_bass()
    return _CACHE["nc"]


def _prep_shards(x, conv_w, kernel_weight):
    x = np.asarray(x, dtype=_F32)
    conv_w = np.asarray(conv_w, dtype=_F32)
    kernel_weight = np.asarray(kernel_weight, dtype=_F32)

    x_pad = np.pad(x, ((0, 0), (0, 0), (1, 1), (1, 1)))
    # lhsT layout: [(kh, c_in), kw, m]
    wt = np.ascontiguousarray(
        conv_w.transpose(2, 1, 3, 0).reshape(96, 3, RANK * C_OUT)
    )
    ident = np.eye(128, dtype=_F32)

    in_maps = []
    for i in range(N_CORES):
        h0 = BAND * i
        shard = np.ascontiguousarray(
            x_pad[:, :, h0 : h0 + ROWS_IN, :]
        ).reshape(B, C_IN, ROWS_IN * WP)
        band = kernel_weight[:, h0 : h0 + BAND, :]          # [2, 32, 256]
        # [r, q, (t j)] where pixel blocks of 512 = 2 rows
        arr = (
            band.reshape(2, SUPER, 2, NBLK)
            .transpose(0, 2, 1, 3)
            .reshape(2, 2, SUPER * NBLK)
        )
        svb = np.broadcast_to(
            arr[:, :, None, :], (2, 2, C_OUT, SUPER * NBLK)
        ).reshape(2, 128, SUPER * NBLK)
        svb = np.ascontiguousarray(svb.transpose(1, 0, 2))  # [128, 2, 4096]
        in_maps.append({"xs": shard, "wt": wt, "svb": svb, "ident": ident})
    return in_maps


def run(inputs, trace=False):
    """Run the sharded bass kernel; returns (out_full, BassKernelResults)."""
    from concourse.bass_utils import run_bass_kernel_spmd

    in_maps = _prep_shards(**inputs)
    nc = _get_bass()
    res = run_bass_kernel_spmd(
        nc, in_maps, core_ids=list(range(N_CORES)), trace=trace
    )
    out = np.empty((B, C_OUT, IMG, IMG), dtype=_F32)
    for i in range(N_CORES):
        out[:, :, BAND * i : BAND * (i + 1), :] = res.results[i]["out"]
    return out, res


def kernel(x, conv_w, kernel_weight):
    out, _ = run({"x": x, "conv_w": conv_w, "kernel_weight": kernel_weight})
    return out
